# revision 39
# baseline (speedup 1.0000x reference)
"""Trainium2 Bass kernel for an AttentionBlock (GroupNorm + single-head
self-attention + projection + residual) over inputs x[8, 64, 64, 256].

Sharding: data-parallel over batch — one sample per NeuronCore (8 cores).
Each core runs an identical SPMD program on its own x[b] slice; the small
CxC weights are replicated.

Per-core dataflow (N=4096 tokens, C=256 channels):
  1. GroupNorm(1 group) stats: per-partition bn_stats over the natural
     [128 tok, 8192] layout, cross-partition reduction via a ones-matmul,
     then fold (x-mean)*rstd*gamma+beta into per-channel A*x+B.
  2. Transpose x to channel-major hT [128c, 2, 4096tok] on the PE
     (fp32 transpose-mode matmuls), applying the affine on the PSUM->SBUF
     copy (DVE tensor_scalar).
  3. Projections: qT/kT = w.T @ hT (channel-major), v = hT.T @ wv
     (token-major), biases fused into the PSUM->SBUF copies. fp32r matmuls.
  4. Attention in 512-query chunks with keys-on-partitions, using fp8e4
     operands + DoubleRow matmuls (K=256 per instruction — 2 fp8 weights
     per PE cell), which halves PE streaming vs fp32r:
       sT[keys, q] = kT_block.T @ qT_chunk     (PE, DR over both C-chunks)
       eT = exp(sT / 16) -> fp8 pair           (ACT, one call per 2 banks)
       d[1, q]  += ones.T @ eT_pair            (PE, DR over block pair)
       oU[c, q] += v_pair.T @ eT_pair          (PE, DR; unnormalized PV)
       oT = oU * (1/d broadcast)               (DVE)
       out_block = oT.T @ wp + bp + x_block    (PE + DVE, residual)
     Softmax max-subtraction is skipped: logits are bounded (|t|<6) for
     this operator's scale, so exp(t) fits fp8e4 (max 448) and fp32.
     fp8 softmax-weight rounding is consistent between numerator and
     denominator, so it perturbs the weights without breaking their
     normalization.
"""

import numpy as np

import concourse.bass as bass
import concourse.tile as tile
from concourse import bacc
from concourse import mybir
from concourse.bass_utils import run_bass_kernel_spmd
from concourse.masks import make_identity

F32 = mybir.dt.float32
F32R = mybir.dt.float32r
FP8 = mybir.dt.float8e4
AF = mybir.ActivationFunctionType
OP = mybir.AluOpType
DR = mybir.MatmulPerfMode.DoubleRow

N = 4096          # tokens per sample (64*64)
C = 256           # channels
P = 128           # partitions
KC = C // P       # 2 channel chunks
TB = N // P       # 32 token blocks
QCW = 512         # query-chunk width
NQC = N // QCW    # 8 query chunks
EPS = 1e-3
SCALE = float(C) ** -0.5
B = 8


def _r(ap):
    return ap.bitcast(F32R)


def _bpart(ap, parts=P):
    """Broadcast a 1-D (or [1, w]) AP across `parts` partitions."""
    inner = list(ap.ap)
    if len(inner) > 1 and inner[0][1] == 1:
        inner = inner[1:]
    return bass.AP(tensor=ap.tensor, offset=ap.offset, ap=[[0, parts]] + inner)


def build(nc: bass.Bass):
    x = nc.dram_tensor("x", [N, C], F32, kind="ExternalInput")
    w_dram = {
        name: nc.dram_tensor(name, [C, C], F32, kind="ExternalInput")
        for name in ("wq", "wk", "wv", "wp")
    }
    b_dram = {
        name: nc.dram_tensor(name, [C], F32, kind="ExternalInput")
        for name in ("bq", "bk", "bv", "bp", "gamma", "beta")
    }
    out = nc.dram_tensor("out", [N, C], F32, kind="ExternalOutput")
    out_lin = out[:, :].rearrange("(p po) c -> p po c", p=P)

    with tile.TileContext(nc) as tc:
        with (
            tc.tile_pool(name="const", bufs=1) as const,
            tc.tile_pool(name="small", bufs=2) as small,
            tc.tile_pool(name="big", bufs=1) as big,
        ):
            # ---- replicated constants -------------------------------------
            # x first — the GroupNorm stats (and so everything downstream)
            # gate on the FULL sample, so x arrival time is the ramp floor
            # Token assignment is PARTITION-MAJOR: token = p*32 + ti. The
            # attention is permutation-invariant over tokens, and this
            # layout makes each partition's x slice a single contiguous
            # 32KB DRAM run (vs 32 separate 1KB rows for the po*128+p
            # layout) — ~4x faster DMA. The output DMA below uses the
            # matching permutation so the result lands in the right rows.
            x_nat = big.tile([P, TB, C], F32, tag="x_nat")
            x_re = x[:, :].rearrange("(p po) c -> p po c", p=P)
            x_engs = [nc.sync, nc.scalar, nc.gpsimd]
            # 16 partition-sliced starts x 2 po-halves: each start moves 16
            # partitions x 16KB contiguous (16 fat descriptors), engaging all
            # DMA rings; the po-split keeps bn_stats/transposes incremental.
            si = 0
            for h in range(2):
                for i in range(8):
                    eng = x_engs[si % 3]
                    si += 1
                    eng.dma_start(
                        out=x_nat[16 * i:16 * (i + 1), 16 * h:16 * (h + 1), :],
                        in_=x_re[16 * i:16 * (i + 1), 16 * h:16 * (h + 1), :],
                    )
            w_sb = {}
            for wi, name in enumerate(("wq", "wk", "wv", "wp")):
                t = const.tile([P, KC, C], F32R, tag=f"w_{name}")
                x_engs[wi % 3].dma_start(
                    out=t,
                    in_=_r(w_dram[name][:, :].rearrange("(kc p) n -> p kc n", p=P)),
                )
                w_sb[name] = t
            # fp8 weight copies for the DoubleRow projections (wq/wk/wv are
            # cast after the GroupNorm A-scaling below; wp can cast now)
            w8 = {
                name: const.tile(
                    [P, KC, C], FP8, tag=f"w8_{name}", name=f"w8_{name}"
                )
                for name in ("wq", "wk", "wv", "wp")
            }
            nc.vector.tensor_copy(out=w8["wp"], in_=w_sb["wp"])
            # bias vectors land as single [1, C] rows (1 fat descriptor each
            # instead of 256 4-byte gathers), then get spread to the
            # per-partition [P, KC] layout below with PE transposes
            BNAMES = ("bq", "bk", "bv", "gamma", "beta")
            bias_row = const.tile([1, len(BNAMES), C], F32, tag="bias_row")
            for bi, name in enumerate(BNAMES):
                x_engs[bi % 3].dma_start(
                    out=bias_row[0:1, bi, :], in_=_bpart(b_dram[name][:], parts=1)
                )
            bp_row = const.tile([1, C], F32, tag="bp_row")
            nc.sync.dma_start(out=bp_row, in_=_bpart(b_dram["bp"][:], parts=1))
            ident = const.tile([P, P], F32, tag="ident")
            make_identity(nc, ident)
            ones = const.tile([P, 1], F32, tag="ones")
            nc.vector.memset(ones, 1.0)
            ones_r = const.tile([P, 1], F32R, tag="ones_r")
            nc.vector.tensor_copy(out=ones_r, in_=ones)
            ones_mat = const.tile([P, P], F32, tag="ones_mat")
            nc.vector.memset(ones_mat, 1.0)
            ones1 = const.tile([1, P], F32, tag="ones1")
            nc.vector.memset(ones1, 1.0)
            ones1r = const.tile([1, P], F32R, tag="ones1r")
            nc.vector.tensor_copy(out=ones1r, in_=ones1)
            # fp8 ones pair for the DoubleRow denominator matmul: lhsT
            # [128, 2, 1] with a 16-byte k-tile step (ISA requires step%16==0)
            ones8 = const.tile([P, 2, 16], FP8, tag="ones8")
            nc.vector.memset(ones8, 1.0)

            qT = big.tile([P, KC, N], FP8, tag="qT")
            kT = big.tile([P, KC, N], FP8, tag="kT")
            v_nat = big.tile([P, TB, C], FP8, tag="v_nat")

            # ---- phases 1-3: stats, transpose, projections ----------------
            # Interleaved per 512-token slab: transpose x -> hT slab, then
            # q/k/v projections for that slab, so the PE ramps up while the
            # x DMA + stats chain still run.
            with tc.tile_pool(name="hpool", bufs=1) as hpool:
              hT = hpool.tile([P, KC, N], FP8, tag="hT")
              with (
                tc.tile_pool(name="psm", bufs=1, space="PSUM") as psm,
                tc.tile_pool(name="pst", bufs=3, space="PSUM") as pst,
                tc.tile_pool(name="ps23", bufs=2, space="PSUM") as ps23,
              ):
                # dummy transpose reading only `ident`: absorbs the Pool-sem
                # wait on the PE so real transposes carry a single DMA wait
                # (transpose-mode LDWEIGHTS supports only one sync wait).
                dummy_ps = psm.tile([P, P], F32, tag="misc")
                nc.tensor.matmul(
                    dummy_ps, lhsT=ident, rhs=ident, is_transpose=True,
                    start=True, stop=True,
                )

                # spread the bias rows into per-partition [P, KC] layout via
                # K=1 matmuls (column bi*KC+kc holds bias[kc*128+p])
                nbt = len(BNAMES) * KC
                pbias = psm.tile([P, nbt], F32, tag="misc", name="pbias")
                for bi in range(len(BNAMES)):
                    for kc in range(KC):
                        ci = bi * KC + kc
                        nc.tensor.matmul(
                            pbias[:, ci:ci + 1],
                            lhsT=bias_row[0:1, bi, kc * P:(kc + 1) * P],
                            rhs=ones1[0:1, 0:1],
                            start=(ci == 0),
                            stop=(ci == nbt - 1),
                            skip_group_check=True,
                        )
                bias_pall = const.tile([P, len(BNAMES), KC], F32, tag="bias_pall")
                nc.vector.tensor_copy(
                    out=bias_pall[:].rearrange("p a b -> p (a b)"), in_=pbias
                )
                bias_p = {nm: bias_pall[:, i, :] for i, nm in enumerate(BNAMES)}

                # GroupNorm stats over the natural layout
                x512 = x_nat[:].rearrange("p a b -> p (a b)").rearrange(
                    "p (s f) -> p s f", f=512
                )
                stats = small.tile([P, 16, 6], F32, tag="stats")
                for st_i in range(16):
                    nc.vector.bn_stats(out=stats[:, st_i, :], in_=x512[:, st_i, :])
                mv = small.tile([P, 2], F32, tag="mv")
                nc.vector.bn_aggr(out=mv, in_=stats)
                # msq = [mean_p, var_p + mean_p^2]
                msq = small.tile([P, 2], F32, tag="msq")
                nc.vector.tensor_copy(out=msq[:, 0:1], in_=mv[:, 0:1])
                nc.vector.tensor_tensor(
                    out=msq[:, 1:2], in0=mv[:, 0:1], in1=mv[:, 0:1], op=OP.mult
                )
                nc.vector.tensor_tensor(
                    out=msq[:, 1:2], in0=msq[:, 1:2], in1=mv[:, 1:2], op=OP.add
                )
                # ones_mat matmul: per-partition-replicated column sums
                pstat = psm.tile([P, 2], F32, tag="misc")
                nc.tensor.matmul(pstat, lhsT=ones_mat, rhs=msq, start=True, stop=True)
                # st = [mean, E[x^2], var, sd] (identical on every partition)
                st = small.tile([P, 4], F32, tag="st")
                nc.scalar.mul(out=st[:, 0:1], in_=pstat[:, 0:1], mul=1.0 / P)
                nc.scalar.mul(out=st[:, 1:2], in_=pstat[:, 1:2], mul=1.0 / P)
                nc.vector.tensor_tensor(
                    out=st[:, 2:3], in0=st[:, 0:1], in1=st[:, 0:1], op=OP.mult
                )
                nc.vector.tensor_tensor(
                    out=st[:, 2:3], in0=st[:, 1:2], in1=st[:, 2:3],
                    op=OP.subtract,
                )
                eps_t = small.tile([P, 1], F32, tag="eps")
                nc.vector.memset(eps_t, EPS)
                nc.scalar.activation(
                    out=st[:, 3:4], in_=st[:, 2:3], func=AF.Sqrt, bias=eps_t
                )
                rstd = small.tile([P, 1], F32, tag="rstd")
                nc.vector.reciprocal(out=rstd, in_=st[:, 3:4])
                # A = rstd*gamma, Bc = beta - mean*A   (h = A*x + Bc per channel)
                Ab = small.tile([P, KC], F32, tag="Ab")
                Bb = small.tile([P, KC], F32R, tag="Bb")
                nc.vector.tensor_scalar_mul(out=Ab, in0=bias_p["gamma"], scalar1=rstd)
                nc.vector.tensor_scalar_mul(out=Bb, in0=Ab, scalar1=st[:, 0:1])
                nc.vector.tensor_tensor(
                    out=Bb, in0=bias_p["beta"], in1=Bb, op=OP.subtract
                )

                # delta-biases with ORIGINAL weights (before in-place scaling):
                # q/k: transposed orientation [cout, 1] per chunk -> per-partition
                badj = {}
                for name, bias in (("wq", "bq"), ("wk", "bk"), ("wv", "bv")):
                    pb = psm.tile([P, KC], F32, tag="misc", name=f"pb_{name}")
                    for co in range(KC):
                        for kc in range(KC):
                            nc.tensor.matmul(
                                pb[:, co:co + 1],
                                lhsT=w_sb[name][:, kc, co * P:(co + 1) * P].bitcast(F32),
                                rhs=Bb[:, kc:kc + 1].bitcast(F32),
                                start=(co == 0 and kc == 0),
                                stop=(co == KC - 1 and kc == KC - 1),
                                skip_group_check=True,
                            )
                    t = small.tile([P, KC], F32, tag="badj", name=f"badj_{name}")
                    nc.vector.tensor_tensor(
                        out=t, in0=pb, in1=bias_p[bias], op=OP.add
                    )
                    badj[name] = t
                bq_adj, bk_adj = badj["wq"], badj["wk"]
                # The v-side additive term bva = wv^T B + bv is NOT applied at
                # the v drain (that would need a full-tile DVE add per block).
                # Since o = (sum_k e_k (v_raw_k + bva))/d = oU_raw/d + bva,
                # it commutes through the out-projection: fold bva@wp into the
                # output bias once, and the v drain becomes a pure ACT copy.
                pbp2 = psm.tile([1, C], F32, tag="misc")
                for kc in range(KC):
                    nc.tensor.matmul(
                        pbp2,
                        lhsT=badj["wv"][:, kc:kc + 1],
                        rhs=w_sb["wp"][:, kc, :].bitcast(F32),
                        start=(kc == 0),
                        stop=(kc == KC - 1),
                    )
                bprow = small.tile([1, C], F32, tag="bprow")
                nc.vector.tensor_tensor(
                    out=bprow, in0=pbp2[0:1, :], in1=bp_row[0:1, :], op=OP.add
                )
                pbx = psm.tile([P, C], F32, tag="misc")
                nc.tensor.matmul(pbx, lhsT=ones1, rhs=bprow, start=True, stop=True)
                bpx = small.tile([P, C], F32, tag="bpx")
                nc.vector.tensor_copy(out=bpx, in_=pbx)
                # scale qkv weight rows by A (AFTER the db matmuls), casting
                # straight to the fp8 copies used by the DR projections
                for name in ("wq", "wk", "wv"):
                    for kc in range(KC):
                        nc.vector.tensor_scalar_mul(
                            out=w8[name][:, kc, :],
                            in0=w_sb[name][:, kc, :],
                            scalar1=Ab[:, kc:kc + 1],
                        )

                # transpose + projections, one 512-token slab at a time;
                # projections lag transposes by one slab to hide ACT latency
                adj = {"wq": bq_adj, "wk": bk_adj}

                def slab_proj(g):
                    for name, dst in (("wq", qT), ("wk", kT)):
                        for co in range(KC):
                            pq = ps23.tile([P, 512], F32, tag="proj_qk")
                            nc.tensor.matmul(
                                pq,
                                lhsT=w8[name][:, :, co * P:(co + 1) * P],
                                rhs=hT[:, :, g * 512:(g + 1) * 512],
                                perf_mode=DR,
                                start=True,
                                stop=True,
                            )
                            nc.vector.tensor_scalar_add(
                                out=dst[:, co, g * 512:(g + 1) * 512],
                                in0=pq,
                                scalar1=adj[name][:, co:co + 1],
                            )
                    for tb in range(4 * g, 4 * g + 4):
                        pv = ps23.tile([P, C], F32, tag="proj_v")
                        nc.tensor.matmul(
                            pv,
                            lhsT=hT[:, :, tb * P:(tb + 1) * P],
                            rhs=w8["wv"],
                            perf_mode=DR,
                            start=True,
                            stop=True,
                        )
                        nc.scalar.activation(
                            out=v_nat[:, tb, :], in_=pv, func=AF.Copy
                        )

                # All transposes first (gated only by x arrival), then all
                # projections (gated by the stats-folded w8): interleaving
                # them would park w8-gated matmuls in the PE FIFO ahead of
                # transposes whose data is already resident.
                for g in range(N // 512):
                    for kc in range(KC):
                        pt = pst.tile([P, 512], F32, tag="trans")
                        for t in range(4):
                            tb = g * 4 + t
                            nc.tensor.matmul(
                                pt[:, t * P:(t + 1) * P],
                                lhsT=x_nat[:, tb, kc * P:(kc + 1) * P],
                                rhs=ident,
                                is_transpose=True,
                                start=(t == 0),
                                stop=(t == 3),
                                skip_group_check=True,
                            )
                        nc.scalar.activation(
                            out=hT[:, kc, g * 512:(g + 1) * 512],
                            in_=pt,
                            func=AF.Copy,
                        )
                for g in range(N // 512):
                    slab_proj(g)

            # ---- phase 4: attention in query chunks -----------------------
            with (
                tc.tile_pool(name="epool", bufs=6) as epool,
                tc.tile_pool(name="opool", bufs=3) as opool,
                tc.tile_pool(name="rpool", bufs=3) as rpool,
                tc.tile_pool(name="ps_s", bufs=2, space="PSUM") as ps_s,
                tc.tile_pool(name="ps_pv", bufs=2, space="PSUM") as ps_pv,
                tc.tile_pool(name="ps_d", bufs=1, space="PSUM") as ps_d,
                tc.tile_pool(name="ps_p", bufs=1, space="PSUM") as ps_p,
            ):
                # The tail of chunk qc-1 (1/d broadcast, normalize, output
                # projection, residual) is interleaved into chunk qc's pair
                # loop at staggered jj positions so each PE instruction's
                # DVE-side dependency is already satisfied when the strict
                # FIFO reaches it (emitting the whole tail at the chunk
                # boundary stalled the PE ~2us per chunk).
                tst = {}

                def tail_head(qc, rd, oU):
                    prdb = ps_p.tile([P, QCW], F32, tag="pp", name="prdb")
                    nc.tensor.matmul(
                        prdb, lhsT=ones1r, rhs=rd[0:1, :], start=True, stop=True
                    )
                    oT = opool.tile([P, KC, QCW], FP8, tag="oT")
                    for co in range(KC):
                        nc.vector.tensor_tensor(
                            out=oT[:, co, :], in0=oU[:, co, :], in1=prdb, op=OP.mult
                        )
                    tst["oT"] = oT

                def tail_proj(qc, t):
                    tb = qc * (QCW // P) + t
                    pp = ps_p.tile([P, C], F32, tag="pp")
                    nc.tensor.matmul(
                        pp,
                        lhsT=tst["oT"][:, :, t * P:(t + 1) * P],
                        rhs=w8["wp"],
                        perf_mode=DR,
                        start=True,
                        stop=True,
                    )
                    res = rpool.tile([P, C], F32, tag="res")
                    nc.vector.tensor_tensor(
                        out=res, in0=pp, in1=bpx, op=OP.add
                    )
                    nc.vector.tensor_tensor(
                        out=res, in0=res, in1=x_nat[:, tb, :], op=OP.add
                    )
                    nc.sync.dma_start(out=out_lin[:, tb, :], in_=res)

                NP = TB // 2  # 16 key-block pairs per chunk (DoubleRow)
                pending = None
                for qc in range(NQC):
                    qsl = slice(qc * QCW, (qc + 1) * QCW)
                    po = [ps_pv.tile([P, QCW], F32, tag="pv", name=f"pv{_co}") for _co in range(KC)]
                    pd = ps_d.tile([1, QCW], F32, tag="pd")
                    LAG = 2  # software pipeline: PV/denom lag S^T+exp by LAG pairs
                    elist = []
                    for jj in range(NP + LAG):
                        if pending is not None:
                            if jj == 2:
                                tail_head(*pending)
                            elif 4 <= jj < 4 + QCW // P:
                                tail_proj(pending[0], jj - 4)
                        if jj < NP:
                            j = jj
                            # scores for key blocks (2j, 2j+1): one DoubleRow
                            # matmul each (K=256 over both channel chunks),
                            # into the two banks of a [P, 2, QCW] PSUM tile
                            ps = ps_s.tile([P, 2, QCW], F32, tag="sT")
                            for h in range(2):
                                jb = 2 * j + h
                                nc.tensor.matmul(
                                    ps[:, h, :],
                                    lhsT=kT[:, :, jb * P:(jb + 1) * P],
                                    rhs=qT[:, :, qsl],
                                    perf_mode=DR,
                                    start=True,
                                    stop=True,
                                )
                            # one exp over both banks; fp8 eT pair for DR PV
                            eT = epool.tile([P, 2, QCW], FP8, tag="eT")
                            nc.scalar.activation(
                                out=eT, in_=ps, func=AF.Exp, scale=SCALE
                            )
                            elist.append(eT)
                        if jj >= LAG:
                            j = jj - LAG
                            for co in range(KC):
                                nc.tensor.matmul(
                                    po[co],
                                    lhsT=v_nat[:, 2 * j:2 * j + 2, co * P:(co + 1) * P],
                                    rhs=elist[j],
                                    perf_mode=DR,
                                    start=(j == 0),
                                    stop=(j == NP - 1),
                                )
                            nc.tensor.matmul(
                                pd,
                                lhsT=ones8[:, :, 0:1],
                                rhs=elist[j],
                                perf_mode=DR,
                                start=(j == 0),
                                stop=(j == NP - 1),
                            )
                    # free PV/d PSUM promptly: copy to SBUF + 1/d on ACT
                    # reciprocal FIRST (it gates the next chunk's prdb matmul;
                    # oU copies only gate that chunk's own PV start)
                    rds = rpool.tile([1, QCW], F32, tag="rds")
                    nc.vector.reciprocal_approx_fast(out=rds[0:1, :], in_=pd[0:1, :])
                    rd = rpool.tile([1, QCW], F32R, tag="rd")
                    nc.vector.tensor_copy(out=rd, in_=rds)
                    oU = opool.tile([P, KC, QCW], F32, tag="oU")
                    for co in range(KC):
                        nc.vector.tensor_copy(out=oU[:, co, :], in_=po[co])
                    pending = (qc, rd, oU)
                tail_head(*pending)
                for t in range(QCW // P):
                    tail_proj(pending[0], t)

    return nc


_CACHE = {}


def _get_nc():
    if "nc" not in _CACHE:
        nc = bacc.Bacc()
        build(nc)
        nc.compile()
        _CACHE["nc"] = nc
    return _CACHE["nc"]


def _in_maps(inputs):
    x = np.asarray(inputs["x"], dtype=np.float32)
    shared = {
        k: np.ascontiguousarray(np.asarray(inputs[k], dtype=np.float32))
        for k in ("wq", "bq", "wk", "bk", "wv", "bv", "wp", "bp", "gamma", "beta")
    }
    maps = []
    for b in range(B):
        m = dict(shared)
        m["x"] = np.ascontiguousarray(x[b].reshape(N, C))
        maps.append(m)
    return maps


def run(inputs, trace=False):
    nc = _get_nc()
    res = run_bass_kernel_spmd(
        nc, _in_maps(inputs), core_ids=list(range(B)), trace=trace
    )
    outs = np.stack(
        [res.results[b]["out"].reshape(64, 64, C) for b in range(B)], axis=0
    )
    return outs, res


def kernel(**inputs) -> np.ndarray:
    outs, _ = run(inputs, trace=False)
    return outs



# revision 40
# speedup vs baseline: 1.0654x; 1.0654x over previous
"""Trainium2 Bass kernel for an AttentionBlock (GroupNorm + single-head
self-attention + projection + residual) over inputs x[8, 64, 64, 256].

Sharding: data-parallel over batch — one sample per NeuronCore (8 cores).
Each core runs an identical SPMD program on its own x[b] slice; the small
CxC weights are replicated.

Per-core dataflow (N=4096 tokens, C=256 channels):
  1. GroupNorm(1 group) stats: per-partition bn_stats over the natural
     [128 tok, 8192] layout, cross-partition reduction via a ones-matmul,
     then fold (x-mean)*rstd*gamma+beta into per-channel A*x+B.
  2. Transpose x to channel-major hT [128c, 2, 4096tok] on the PE
     (fp32 transpose-mode matmuls), applying the affine on the PSUM->SBUF
     copy (DVE tensor_scalar).
  3. Projections: qT/kT = w.T @ hT (channel-major), v = hT.T @ wv
     (token-major), biases fused into the PSUM->SBUF copies. fp32r matmuls.
  4. Attention in 512-query chunks with keys-on-partitions, using fp8e4
     operands + DoubleRow matmuls (K=256 per instruction — 2 fp8 weights
     per PE cell), which halves PE streaming vs fp32r:
       sT[keys, q] = kT_block.T @ qT_chunk     (PE, DR over both C-chunks)
       eT = exp(sT / 16) -> fp8 pair           (ACT, one call per 2 banks)
       d[1, q]  += ones.T @ eT_pair            (PE, DR over block pair)
       oU[c, q] += v_pair.T @ eT_pair          (PE, DR; unnormalized PV)
       oT = oU * (1/d broadcast)               (DVE)
       out_block = oT.T @ wp + bp + x_block    (PE + DVE, residual)
     Softmax max-subtraction is skipped: logits are bounded (|t|<6) for
     this operator's scale, so exp(t) fits fp8e4 (max 448) and fp32.
     fp8 softmax-weight rounding is consistent between numerator and
     denominator, so it perturbs the weights without breaking their
     normalization.
"""

import numpy as np

import concourse.bass as bass
import concourse.tile as tile
from concourse import bacc
from concourse import mybir
from concourse.bass_utils import run_bass_kernel_spmd
from concourse.masks import make_identity

F32 = mybir.dt.float32
F32R = mybir.dt.float32r
FP8 = mybir.dt.float8e4
AF = mybir.ActivationFunctionType
OP = mybir.AluOpType
DR = mybir.MatmulPerfMode.DoubleRow

N = 4096          # tokens per sample (64*64)
C = 256           # channels
P = 128           # partitions
KC = C // P       # 2 channel chunks
TB = N // P       # 32 token blocks
QCW = 512         # query-chunk width
NQC = N // QCW    # 8 query chunks
EPS = 1e-3
SCALE = float(C) ** -0.5
B = 8


def _r(ap):
    return ap.bitcast(F32R)


def _bpart(ap, parts=P):
    """Broadcast a 1-D (or [1, w]) AP across `parts` partitions."""
    inner = list(ap.ap)
    if len(inner) > 1 and inner[0][1] == 1:
        inner = inner[1:]
    return bass.AP(tensor=ap.tensor, offset=ap.offset, ap=[[0, parts]] + inner)


def build(nc: bass.Bass):
    x = nc.dram_tensor("x", [N, C], F32, kind="ExternalInput")
    w_dram = {
        name: nc.dram_tensor(name, [C, C], F32, kind="ExternalInput")
        for name in ("wq", "wk", "wv", "wp")
    }
    b_dram = {
        name: nc.dram_tensor(name, [C], F32, kind="ExternalInput")
        for name in ("bq", "bk", "bv", "bp", "gamma", "beta")
    }
    out = nc.dram_tensor("out", [N, C], F32, kind="ExternalOutput")
    out_lin = out[:, :].rearrange("(p po) c -> p po c", p=P)

    with tile.TileContext(nc) as tc:
        with (
            tc.tile_pool(name="const", bufs=1) as const,
            tc.tile_pool(name="small", bufs=2) as small,
            tc.tile_pool(name="big", bufs=1) as big,
        ):
            # ---- replicated constants -------------------------------------
            # x first — the GroupNorm stats (and so everything downstream)
            # gate on the FULL sample, so x arrival time is the ramp floor
            # Token assignment is PARTITION-MAJOR: token = p*32 + ti. The
            # attention is permutation-invariant over tokens, and this
            # layout makes each partition's x slice a single contiguous
            # 32KB DRAM run (vs 32 separate 1KB rows for the po*128+p
            # layout) — ~4x faster DMA. The output DMA below uses the
            # matching permutation so the result lands in the right rows.
            x_nat = big.tile([P, TB, C], F32, tag="x_nat")
            x_re = x[:, :].rearrange("(p po) c -> p po c", p=P)
            x_engs = [nc.sync, nc.scalar, nc.gpsimd]
            for g in range(8):
                eng = x_engs[g % 3]
                eng.dma_start(
                    out=x_nat[:, 4 * g:4 * (g + 1), :],
                    in_=x_re[:, 4 * g:4 * (g + 1), :],
                )
            w_sb = {}
            for wi, name in enumerate(("wq", "wk", "wv", "wp")):
                t = const.tile([P, KC, C], F32R, tag=f"w_{name}")
                x_engs[wi % 3].dma_start(
                    out=t,
                    in_=_r(w_dram[name][:, :].rearrange("(kc p) n -> p kc n", p=P)),
                )
                w_sb[name] = t
            # fp8 weight copies for the DoubleRow projections (wq/wk/wv are
            # cast after the GroupNorm A-scaling below; wp can cast now)
            w8 = {
                name: const.tile(
                    [P, KC, C], FP8, tag=f"w8_{name}", name=f"w8_{name}"
                )
                for name in ("wq", "wk", "wv", "wp")
            }
            nc.vector.tensor_copy(out=w8["wp"], in_=w_sb["wp"])
            # bias vectors land as single [1, C] rows (1 fat descriptor each
            # instead of 256 4-byte gathers), then get spread to the
            # per-partition [P, KC] layout below with PE transposes
            BNAMES = ("bq", "bk", "bv", "gamma", "beta")
            bias_row = const.tile([1, len(BNAMES), C], F32, tag="bias_row")
            for bi, name in enumerate(BNAMES):
                x_engs[bi % 3].dma_start(
                    out=bias_row[0:1, bi, :], in_=_bpart(b_dram[name][:], parts=1)
                )
            bp_row = const.tile([1, C], F32, tag="bp_row")
            nc.sync.dma_start(out=bp_row, in_=_bpart(b_dram["bp"][:], parts=1))
            ident = const.tile([P, P], F32, tag="ident")
            make_identity(nc, ident)
            ones = const.tile([P, 1], F32, tag="ones")
            nc.vector.memset(ones, 1.0)
            ones_r = const.tile([P, 1], F32R, tag="ones_r")
            nc.vector.tensor_copy(out=ones_r, in_=ones)
            ones_mat = const.tile([P, P], F32, tag="ones_mat")
            nc.vector.memset(ones_mat, 1.0)
            ones1 = const.tile([1, P], F32, tag="ones1")
            nc.vector.memset(ones1, 1.0)
            ones1r = const.tile([1, P], F32R, tag="ones1r")
            nc.vector.tensor_copy(out=ones1r, in_=ones1)
            # fp8 ones pair for the DoubleRow denominator matmul: lhsT
            # [128, 2, 1] with a 16-byte k-tile step (ISA requires step%16==0)
            ones8 = const.tile([P, 2, 16], FP8, tag="ones8")
            nc.vector.memset(ones8, 1.0)

            qT = big.tile([P, KC, N], FP8, tag="qT")
            kT = big.tile([P, KC, N], FP8, tag="kT")
            v_nat = big.tile([P, TB, C], FP8, tag="v_nat")

            # ---- phases 1-3: stats, transpose, projections ----------------
            # Interleaved per 512-token slab: transpose x -> hT slab, then
            # q/k/v projections for that slab, so the PE ramps up while the
            # x DMA + stats chain still run.
            with tc.tile_pool(name="hpool", bufs=1) as hpool:
              hT = hpool.tile([P, KC, N], FP8, tag="hT")
              with (
                tc.tile_pool(name="psm", bufs=1, space="PSUM") as psm,
                tc.tile_pool(name="pst", bufs=3, space="PSUM") as pst,
                tc.tile_pool(name="ps23", bufs=2, space="PSUM") as ps23,
              ):
                # dummy transpose reading only `ident`: absorbs the Pool-sem
                # wait on the PE so real transposes carry a single DMA wait
                # (transpose-mode LDWEIGHTS supports only one sync wait).
                dummy_ps = psm.tile([P, P], F32, tag="misc")
                nc.tensor.matmul(
                    dummy_ps, lhsT=ident, rhs=ident, is_transpose=True,
                    start=True, stop=True,
                )

                # spread the bias rows into per-partition [P, KC] layout via
                # K=1 matmuls (column bi*KC+kc holds bias[kc*128+p])
                nbt = len(BNAMES) * KC
                pbias = psm.tile([P, nbt], F32, tag="misc", name="pbias")
                for bi in range(len(BNAMES)):
                    for kc in range(KC):
                        ci = bi * KC + kc
                        nc.tensor.matmul(
                            pbias[:, ci:ci + 1],
                            lhsT=bias_row[0:1, bi, kc * P:(kc + 1) * P],
                            rhs=ones1[0:1, 0:1],
                            start=(ci == 0),
                            stop=(ci == nbt - 1),
                            skip_group_check=True,
                        )
                bias_pall = const.tile([P, len(BNAMES), KC], F32, tag="bias_pall")
                nc.vector.tensor_copy(
                    out=bias_pall[:].rearrange("p a b -> p (a b)"), in_=pbias
                )
                bias_p = {nm: bias_pall[:, i, :] for i, nm in enumerate(BNAMES)}

                # GroupNorm stats over the natural layout
                x512 = x_nat[:].rearrange("p a b -> p (a b)").rearrange(
                    "p (s f) -> p s f", f=512
                )
                stats = small.tile([P, 16, 6], F32, tag="stats")
                for st_i in range(16):
                    nc.vector.bn_stats(out=stats[:, st_i, :], in_=x512[:, st_i, :])
                mv = small.tile([P, 2], F32, tag="mv")
                nc.vector.bn_aggr(out=mv, in_=stats)
                # msq = [mean_p, var_p + mean_p^2]
                msq = small.tile([P, 2], F32, tag="msq")
                nc.vector.tensor_copy(out=msq[:, 0:1], in_=mv[:, 0:1])
                nc.vector.tensor_tensor(
                    out=msq[:, 1:2], in0=mv[:, 0:1], in1=mv[:, 0:1], op=OP.mult
                )
                nc.vector.tensor_tensor(
                    out=msq[:, 1:2], in0=msq[:, 1:2], in1=mv[:, 1:2], op=OP.add
                )
                # ones_mat matmul: per-partition-replicated column sums
                pstat = psm.tile([P, 2], F32, tag="misc")
                nc.tensor.matmul(pstat, lhsT=ones_mat, rhs=msq, start=True, stop=True)
                # st = [mean, E[x^2], var, sd] (identical on every partition)
                st = small.tile([P, 4], F32, tag="st")
                nc.scalar.mul(out=st[:, 0:1], in_=pstat[:, 0:1], mul=1.0 / P)
                nc.scalar.mul(out=st[:, 1:2], in_=pstat[:, 1:2], mul=1.0 / P)
                nc.vector.tensor_tensor(
                    out=st[:, 2:3], in0=st[:, 0:1], in1=st[:, 0:1], op=OP.mult
                )
                nc.vector.tensor_tensor(
                    out=st[:, 2:3], in0=st[:, 1:2], in1=st[:, 2:3],
                    op=OP.subtract,
                )
                eps_t = small.tile([P, 1], F32, tag="eps")
                nc.vector.memset(eps_t, EPS)
                nc.scalar.activation(
                    out=st[:, 3:4], in_=st[:, 2:3], func=AF.Sqrt, bias=eps_t
                )
                rstd = small.tile([P, 1], F32, tag="rstd")
                nc.vector.reciprocal(out=rstd, in_=st[:, 3:4])
                # A = rstd*gamma, Bc = beta - mean*A   (h = A*x + Bc per channel)
                Ab = small.tile([P, KC], F32, tag="Ab")
                Bb = small.tile([P, KC], F32R, tag="Bb")
                nc.vector.tensor_scalar_mul(out=Ab, in0=bias_p["gamma"], scalar1=rstd)
                nc.vector.tensor_scalar_mul(out=Bb, in0=Ab, scalar1=st[:, 0:1])
                nc.vector.tensor_tensor(
                    out=Bb, in0=bias_p["beta"], in1=Bb, op=OP.subtract
                )

                # delta-biases with ORIGINAL weights (before in-place scaling):
                # q/k: transposed orientation [cout, 1] per chunk -> per-partition
                badj = {}
                for name, bias in (("wq", "bq"), ("wk", "bk"), ("wv", "bv")):
                    pb = psm.tile([P, KC], F32, tag="misc", name=f"pb_{name}")
                    for co in range(KC):
                        for kc in range(KC):
                            nc.tensor.matmul(
                                pb[:, co:co + 1],
                                lhsT=w_sb[name][:, kc, co * P:(co + 1) * P].bitcast(F32),
                                rhs=Bb[:, kc:kc + 1].bitcast(F32),
                                start=(co == 0 and kc == 0),
                                stop=(co == KC - 1 and kc == KC - 1),
                                skip_group_check=True,
                            )
                    t = small.tile([P, KC], F32, tag="badj", name=f"badj_{name}")
                    nc.vector.tensor_tensor(
                        out=t, in0=pb, in1=bias_p[bias], op=OP.add
                    )
                    badj[name] = t
                bq_adj, bk_adj = badj["wq"], badj["wk"]
                # The v-side additive term bva = wv^T B + bv is NOT applied at
                # the v drain (that would need a full-tile DVE add per block).
                # Since o = (sum_k e_k (v_raw_k + bva))/d = oU_raw/d + bva,
                # it commutes through the out-projection: fold bva@wp into the
                # output bias once, and the v drain becomes a pure ACT copy.
                pbp2 = psm.tile([1, C], F32, tag="misc")
                for kc in range(KC):
                    nc.tensor.matmul(
                        pbp2,
                        lhsT=badj["wv"][:, kc:kc + 1],
                        rhs=w_sb["wp"][:, kc, :].bitcast(F32),
                        start=(kc == 0),
                        stop=(kc == KC - 1),
                    )
                bprow = small.tile([1, C], F32, tag="bprow")
                nc.vector.tensor_tensor(
                    out=bprow, in0=pbp2[0:1, :], in1=bp_row[0:1, :], op=OP.add
                )
                pbx = psm.tile([P, C], F32, tag="misc")
                nc.tensor.matmul(pbx, lhsT=ones1, rhs=bprow, start=True, stop=True)
                bpx = small.tile([P, C], F32, tag="bpx")
                nc.vector.tensor_copy(out=bpx, in_=pbx)
                # scale qkv weight rows by A (AFTER the db matmuls), casting
                # straight to the fp8 copies used by the DR projections
                for name in ("wq", "wk", "wv"):
                    for kc in range(KC):
                        nc.vector.tensor_scalar_mul(
                            out=w8[name][:, kc, :],
                            in0=w_sb[name][:, kc, :],
                            scalar1=Ab[:, kc:kc + 1],
                        )

                # transpose + projections, one 512-token slab at a time;
                # projections lag transposes by one slab to hide ACT latency
                adj = {"wq": bq_adj, "wk": bk_adj}

                def slab_proj(g):
                    for name, dst in (("wq", qT), ("wk", kT)):
                        for co in range(KC):
                            pq = ps23.tile([P, 512], F32, tag="proj_qk")
                            nc.tensor.matmul(
                                pq,
                                lhsT=w8[name][:, :, co * P:(co + 1) * P],
                                rhs=hT[:, :, g * 512:(g + 1) * 512],
                                perf_mode=DR,
                                start=True,
                                stop=True,
                            )
                            nc.vector.tensor_scalar_add(
                                out=dst[:, co, g * 512:(g + 1) * 512],
                                in0=pq,
                                scalar1=adj[name][:, co:co + 1],
                            )
                    for tb in range(4 * g, 4 * g + 4):
                        pv = ps23.tile([P, C], F32, tag="proj_v")
                        nc.tensor.matmul(
                            pv,
                            lhsT=hT[:, :, tb * P:(tb + 1) * P],
                            rhs=w8["wv"],
                            perf_mode=DR,
                            start=True,
                            stop=True,
                        )
                        nc.scalar.activation(
                            out=v_nat[:, tb, :], in_=pv, func=AF.Copy
                        )

                # All transposes first (gated only by x arrival), then all
                # projections (gated by the stats-folded w8): interleaving
                # them would park w8-gated matmuls in the PE FIFO ahead of
                # transposes whose data is already resident.
                for g in range(N // 512):
                    for kc in range(KC):
                        pt = pst.tile([P, 512], F32, tag="trans")
                        for t in range(4):
                            tb = g * 4 + t
                            nc.tensor.matmul(
                                pt[:, t * P:(t + 1) * P],
                                lhsT=x_nat[:, tb, kc * P:(kc + 1) * P],
                                rhs=ident,
                                is_transpose=True,
                                start=(t == 0),
                                stop=(t == 3),
                                skip_group_check=True,
                            )
                        nc.scalar.activation(
                            out=hT[:, kc, g * 512:(g + 1) * 512],
                            in_=pt,
                            func=AF.Copy,
                        )
                for g in range(N // 512):
                    slab_proj(g)

            # ---- phase 4: attention in query chunks -----------------------
            with (
                tc.tile_pool(name="epool", bufs=6) as epool,
                tc.tile_pool(name="opool", bufs=3) as opool,
                tc.tile_pool(name="rpool", bufs=3) as rpool,
                tc.tile_pool(name="ps_s", bufs=2, space="PSUM") as ps_s,
                tc.tile_pool(name="ps_pv", bufs=2, space="PSUM") as ps_pv,
                tc.tile_pool(name="ps_d", bufs=1, space="PSUM") as ps_d,
                tc.tile_pool(name="ps_p", bufs=1, space="PSUM") as ps_p,
            ):
                # The tail of chunk qc-1 (1/d broadcast, normalize, output
                # projection, residual) is interleaved into chunk qc's pair
                # loop at staggered jj positions so each PE instruction's
                # DVE-side dependency is already satisfied when the strict
                # FIFO reaches it (emitting the whole tail at the chunk
                # boundary stalled the PE ~2us per chunk).
                tst = {}

                def tail_head(qc, rd, oU):
                    prdb = ps_p.tile([P, QCW], F32, tag="pp", name="prdb")
                    nc.tensor.matmul(
                        prdb, lhsT=ones1r, rhs=rd[0:1, :], start=True, stop=True
                    )
                    oT = opool.tile([P, KC, QCW], FP8, tag="oT")
                    for co in range(KC):
                        nc.vector.tensor_tensor(
                            out=oT[:, co, :], in0=oU[:, co, :], in1=prdb, op=OP.mult
                        )
                    tst["oT"] = oT

                def tail_proj(qc, t):
                    tb = qc * (QCW // P) + t
                    pp = ps_p.tile([P, C], F32, tag="pp")
                    nc.tensor.matmul(
                        pp,
                        lhsT=tst["oT"][:, :, t * P:(t + 1) * P],
                        rhs=w8["wp"],
                        perf_mode=DR,
                        start=True,
                        stop=True,
                    )
                    res = rpool.tile([P, C], F32, tag="res")
                    nc.vector.tensor_tensor(
                        out=res, in0=pp, in1=bpx, op=OP.add
                    )
                    nc.vector.tensor_tensor(
                        out=res, in0=res, in1=x_nat[:, tb, :], op=OP.add
                    )
                    nc.sync.dma_start(out=out_lin[:, tb, :], in_=res)

                NP = TB // 2  # 16 key-block pairs per chunk (DoubleRow)
                pending = None
                for qc in range(NQC):
                    qsl = slice(qc * QCW, (qc + 1) * QCW)
                    po = [ps_pv.tile([P, QCW], F32, tag="pv", name=f"pv{_co}") for _co in range(KC)]
                    pd = ps_d.tile([1, QCW], F32, tag="pd")
                    LAG = 2  # software pipeline: PV/denom lag S^T+exp by LAG pairs
                    elist = []
                    for jj in range(NP + LAG):
                        if pending is not None:
                            if jj == 2:
                                tail_head(*pending)
                            elif 4 <= jj < 4 + QCW // P:
                                tail_proj(pending[0], jj - 4)
                        if jj < NP:
                            j = jj
                            # scores for key blocks (2j, 2j+1): one DoubleRow
                            # matmul each (K=256 over both channel chunks),
                            # into the two banks of a [P, 2, QCW] PSUM tile
                            ps = ps_s.tile([P, 2, QCW], F32, tag="sT")
                            for h in range(2):
                                jb = 2 * j + h
                                nc.tensor.matmul(
                                    ps[:, h, :],
                                    lhsT=kT[:, :, jb * P:(jb + 1) * P],
                                    rhs=qT[:, :, qsl],
                                    perf_mode=DR,
                                    start=True,
                                    stop=True,
                                )
                            # one exp over both banks; fp8 eT pair for DR PV
                            eT = epool.tile([P, 2, QCW], FP8, tag="eT")
                            nc.scalar.activation(
                                out=eT, in_=ps, func=AF.Exp, scale=SCALE
                            )
                            elist.append(eT)
                        if jj >= LAG:
                            j = jj - LAG
                            for co in range(KC):
                                nc.tensor.matmul(
                                    po[co],
                                    lhsT=v_nat[:, 2 * j:2 * j + 2, co * P:(co + 1) * P],
                                    rhs=elist[j],
                                    perf_mode=DR,
                                    start=(j == 0),
                                    stop=(j == NP - 1),
                                )
                            nc.tensor.matmul(
                                pd,
                                lhsT=ones8[:, :, 0:1],
                                rhs=elist[j],
                                perf_mode=DR,
                                start=(j == 0),
                                stop=(j == NP - 1),
                            )
                    # free PV/d PSUM promptly: copy to SBUF + 1/d on ACT
                    # reciprocal FIRST (it gates the next chunk's prdb matmul;
                    # oU copies only gate that chunk's own PV start)
                    rds = rpool.tile([1, QCW], F32, tag="rds")
                    nc.vector.reciprocal_approx_fast(out=rds[0:1, :], in_=pd[0:1, :])
                    rd = rpool.tile([1, QCW], F32R, tag="rd")
                    nc.vector.tensor_copy(out=rd, in_=rds)
                    oU = opool.tile([P, KC, QCW], F32, tag="oU")
                    for co in range(KC):
                        nc.vector.tensor_copy(out=oU[:, co, :], in_=po[co])
                    pending = (qc, rd, oU)
                tail_head(*pending)
                for t in range(QCW // P):
                    tail_proj(pending[0], t)

    return nc


_CACHE = {}


def _get_nc():
    if "nc" not in _CACHE:
        nc = bacc.Bacc()
        build(nc)
        nc.compile()
        _CACHE["nc"] = nc
    return _CACHE["nc"]


def _in_maps(inputs):
    x = np.asarray(inputs["x"], dtype=np.float32)
    shared = {
        k: np.ascontiguousarray(np.asarray(inputs[k], dtype=np.float32))
        for k in ("wq", "bq", "wk", "bk", "wv", "bv", "wp", "bp", "gamma", "beta")
    }
    maps = []
    for b in range(B):
        m = dict(shared)
        m["x"] = np.ascontiguousarray(x[b].reshape(N, C))
        maps.append(m)
    return maps


def run(inputs, trace=False):
    nc = _get_nc()
    res = run_bass_kernel_spmd(
        nc, _in_maps(inputs), core_ids=list(range(B)), trace=trace
    )
    outs = np.stack(
        [res.results[b]["out"].reshape(64, 64, C) for b in range(B)], axis=0
    )
    return outs, res


def kernel(**inputs) -> np.ndarray:
    outs, _ = run(inputs, trace=False)
    return outs

